# revision 33
# baseline (speedup 1.0000x reference)
"""H2GCN forward on 8 Trainium2 NeuronCores — dense fp8 DoubleRow SpMM.

out = concat([h0, A1@h0, A2@h0], 1) @ W_out + b_out,  h0 = x @ W1

Data-parallel over destination nodes (1250 rows/core). Per core:
  h0 = x_shard @ W1 in bf16 (t-major, xT k-tiles streamed from DRAM),
  AllGather h0 in fp8e4 (two halves: rows 0-511 / 512-1249),
  SpMM as dense-block matmuls in fp8 DoubleRow perf mode: each instruction
  contracts a PAIR of 128-row source tiles (256 rows) against the dest-tile
  selection block at 0.5 cycles/row — 4x the bf16 dense rate, and the A
  matrices are 26 MB instead of 52 MB.  Half-0/half-1 source partials are
  kept separate (hT k-slices 2-5 / 6-9) so half-0 matmuls run while the
  second AllGather is still in flight; the final out GEMM sums both.
"""
import sys
import types

for _p in ("/opt/trn_rl_repo", "/root/.axon_site", "/root/.axon_site/_ro/trn_rl_repo",
           "/root/.axon_site/_ro/pypackages"):
    if _p not in sys.path:
        sys.path.append(_p)

import numpy as np
import ml_dtypes
import concourse.bass as bass
import concourse.bacc as bacc
import concourse.mybir as mybir
import concourse.tile as tile
from concourse import bass_utils

N, IN_C, HID, OUT_C = 10000, 2048, 256, 256
NCORES = 8
ROWS = N // NCORES          # 1250
NT = 10                     # dest tiles of 128 (last has 98 valid rows)
KT = IN_C // 128            # 16
H0T = 4                     # AG half 0: source tiles 0-3 of each shard
H1T = 6                     # half 1: tiles 4-9 incl. zero pad rows 1250-1279
PAIRS = ([0, 1], [2, 3, 4])  # DoubleRow source-tile pairs per AG half
NPAIR = 5                   # source-tile pairs per shard (DoubleRow)
WV = 64                     # chunks per A-stream wave (16KB/partition descs)

f32 = mybir.dt.float32
bf16 = mybir.dt.bfloat16
f8 = mybir.dt.float8e4
bfnp = ml_dtypes.bfloat16
f8np = ml_dtypes.float8_e4m3

# small weights blob (bf16): Wout k-tiles, bias, ones, identity
OWO, OB, OO, OI = 0, 6 * OUT_C, 6 * OUT_C + OUT_C, 6 * OUT_C + OUT_C + 128
BLOBW = OI + 128

LAST_EXEC_NS = None
LAST_RESULTS = None


def _install_trace_shim():
    try:
        import antenv.axon_hooks  # noqa: F401
        return
    except ImportError:
        pass
    try:
        import antenv
        from trn_agent_boot.trn_boot import _ntff_profile_via_ctypes
        hook = _ntff_profile_via_ctypes("/opt/axon/libaxon_pjrt.so")
        mod = types.ModuleType("antenv.axon_hooks")
        mod.get_axon_ntff_profile_hook = lambda: hook
        mod.set_axon_ntff_profile_hook = lambda h: None
        sys.modules["antenv.axon_hooks"] = mod
        antenv.axon_hooks = mod
    except Exception:
        pass


def _chunk_meta():
    """Global chunk order: half-major, dest-tile, adj, then (core, pair).
    half 0 = pairs 0-1 (src rows 0-511), half 1 = pairs 2-4 (rows 512-1279).
    Returns (meta, tile_last): meta[i] = (a, t, h, first, last)."""
    meta = []
    tile_last = {}
    for h in (0, 1):
        pairs = PAIRS[h]
        for t in range(NT):
            for a in (0, 1):
                n = NCORES * len(pairs)
                for i in range(n):
                    meta.append((a, t, h, i == 0, i == n - 1))
                    tile_last[t] = len(meta) - 1
    return meta, tile_last


def _prep_xt(x_bf, core):
    """xT blob: block (t, k) at cols (t*KT + k)*128, [128 feat, 128 rows]."""
    xt = np.zeros((128, NT * KT * 128), bfnp)
    xsh = x_bf[core * ROWS:(core + 1) * ROWS]
    for t in range(NT):
        rows = min(128, ROWS - t * 128)
        b = xsh[t * 128:t * 128 + rows].T.reshape(KT, 128, rows)
        for k in range(KT):
            xt[:, (t * KT + k) * 128:(t * KT + k) * 128 + rows] = b[k]
    return xt


def _prep_adj(adj, core):
    """Dense A^T per adjacency for this core's dest shard in fp8, laid out in
    global chunk order: chunk = (a,t,h,core r,pair j2) -> [128, 2, 128]."""
    dense = []
    for (rows, cols, vals) in adj:
        lo = core * ROWS
        m = (rows >= lo) & (rows < lo + ROWS)
        r, c, v = rows[m] - lo, cols[m], vals[m]
        # src index in padded tile space: core*1280 + (col % 1250)
        src = (c // ROWS) * (NT * 128) + (c % ROWS)
        A = np.zeros((NCORES * NT * 128, ROWS), np.float32)
        np.add.at(A, (src, r), v)
        dense.append(A.astype(f8np))
    meta, _ = _chunk_meta()
    blob = np.zeros((128, len(meta) * 256), f8np)
    pos = {}
    cnt = {}
    for i, (a, t, h, _, _) in enumerate(meta):
        k = (a, t, h)
        j = cnt.get(k, 0)
        cnt[k] = j + 1
        pos[(a, t, h, j)] = i
    for a in (0, 1):
        A = dense[a]
        for h in (0, 1):
            pairs = PAIRS[h]
            for t in range(NT):
                for rj, (rr, j2) in enumerate(
                        (rr, j2) for rr in range(NCORES) for j2 in pairs):
                    i = pos[(a, t, h, rj)]
                    for half_pair in range(2):
                        s = rr * NT + 2 * j2 + half_pair
                        blk = A[s * 128:(s + 1) * 128,
                                t * 128:min((t + 1) * 128, ROWS)]
                        blob[:, i * 256 + half_pair * 128:
                             i * 256 + half_pair * 128 + blk.shape[1]] = blk
    return blob


def _build():
    meta, tile_last = _chunk_meta()
    nchunks = len(meta)
    nc = bacc.Bacc("TRN2", target_bir_lowering=False, debug=False,
                   num_devices=NCORES)
    xt_d = nc.dram_tensor("xt", [128, NT * KT * 128], bf16, kind="ExternalInput")
    w1_d = nc.dram_tensor("w1", [128, KT * HID], bf16, kind="ExternalInput")
    ws_d = nc.dram_tensor("ws", [128, BLOBW], bf16, kind="ExternalInput")
    a_d = nc.dram_tensor("ablob", [128, nchunks * 256], f8, kind="ExternalInput")
    out = nc.dram_tensor("out", [ROWS, OUT_C], f32, kind="ExternalOutput")

    with tile.TileContext(nc) as tc:
        with tc.tile_pool(name="keep", bufs=1) as keep, \
             tc.tile_pool(name="dram", bufs=1, space="DRAM") as dram, \
             tc.tile_pool(name="pmm", bufs=2, space="PSUM") as pmm, \
             tc.tile_pool(name="psm", bufs=3, space="PSUM") as psm, \
             tc.tile_pool(name="ptr", bufs=2, space="PSUM") as ptr:

            w1_sb = keep.tile([128, KT * HID], bf16)
            ws_sb = keep.tile([128, BLOBW], bf16)
            ag_sb = keep.tile([128, NT, HID], f8)
            h0a = keep.tile([128, NCORES * NT, HID], f8)
            # hT k-slices: 0,1 = h0; 2..5 = h1,h2 half-0 partials;
            # 6..9 = h1,h2 half-1 partials (out GEMM sums both)
            hT = keep.tile([128, 10, NT * 128], bf16)
            h12 = keep.tile([128, 2, NT, HID], bf16)
            nc.sync.dma_start(w1_sb[:], w1_d[:])
            nc.sync.dma_start(ws_sb[:], ws_d[:])

            # partition-major AG payload: row p = SBUF partition p's tiles,
            # so every DMA on both sides is contiguous (1-1.5KB descriptors)
            ag_in0 = dram.tile([128, H0T * HID], f8)
            ag_in1 = dram.tile([128, H1T * HID], f8)
            ag_out0 = dram.tile([NCORES * 128, H0T * HID], f8, addr_space="Shared")
            ag_out1 = dram.tile([NCORES * 128, H1T * HID], f8, addr_space="Shared")
            h0bf = keep.tile([128, NT, HID], bf16)

            # ---- phase A: h0 = x @ W1 (bf16), 4-tile xT chunks (16KB descs) ----
            with nc.named_scope("h0_gemm"):
                xch = {}
                for t in range(NT):
                    c = t // 4
                    if t % 4 == 0:
                        n_t = min(4, NT - c * 4)
                        xch[c] = keep.tile([128, 4 * KT * 128], bf16,
                                           tag="xt", bufs=2, name="xtc")
                        nc.sync.dma_start(
                            xch[c][:, 0:n_t * KT * 128],
                            xt_d[:, c * 4 * KT * 128:
                                 (c * 4 + n_t) * KT * 128])
                    base = (t % 4) * KT * 128
                    ps = pmm.tile([128, HID], f32, tag="mm")
                    for k in range(KT):
                        nc.tensor.matmul(
                            ps[:],
                            xch[c][:, base + k * 128:base + (k + 1) * 128],
                            w1_sb[:, k * HID:(k + 1) * HID],
                            start=(k == 0), stop=(k == KT - 1))
                    nc.vector.tensor_copy(h0bf[:, t, :], ps[:])
                    nc.vector.tensor_copy(ag_sb[:, t, :], ps[:])
                    if t == H0T - 1:
                        nc.scalar.dma_start(
                            ag_in0[:],
                            ag_sb[:, 0:H0T, :].rearrange("p t m -> p (t m)"))
                    if t == NT - 1:
                        nc.scalar.dma_start(
                            ag_in1[:],
                            ag_sb[:, H0T:NT, :].rearrange("p t m -> p (t m)"))

            # ---- phase B: AllGather h0 (fp8), two halves ----
            with nc.named_scope("allgather"):
                nc.gpsimd.collective_compute(
                    "AllGather", mybir.AluOpType.bypass,
                    replica_groups=[list(range(NCORES))],
                    ins=[ag_in0.opt()], outs=[ag_out0.opt()])
                nc.gpsimd.collective_compute(
                    "AllGather", mybir.AluOpType.bypass,
                    replica_groups=[list(range(NCORES))],
                    ins=[ag_in1.opt()], outs=[ag_out1.opt()])

            # ---- deferred h0 transposes: run on PE during the AG wait ----
            with nc.named_scope("h0t"):
                for t in range(NT):
                    for hf in range(2):
                        pst = ptr.tile([128, 128], bf16, tag="tr")
                        nc.tensor.transpose(
                            pst[:], h0bf[:, t, 128 * hf:128 * (hf + 1)],
                            ws_sb[:, OI:OI + 128])
                        nc.vector.tensor_copy(
                            hT[:, hf, 128 * t:128 * (t + 1)], pst[:])

            # ---- phase C: dense SpMM, fp8 DoubleRow, streamed A ----
            with nc.named_scope("spmm"):
                cur_ps = {}
                srcpair = {}
                cnt = {}
                for i, (a, t, h, _, _) in enumerate(meta):
                    pairs = PAIRS[h]
                    j = cnt.get((a, t, h), 0)
                    cnt[(a, t, h)] = j + 1
                    rr, j2 = j // len(pairs), pairs[j % len(pairs)]
                    srcpair[i] = rr * NPAIR + j2
                # wave boundaries (don't cross the half boundary)
                wavespans = []
                ci = 0
                while ci < len(meta):
                    wn = min(WV, len(meta) - ci)
                    hcur = meta[ci][2]
                    while meta[ci + wn - 1][2] != hcur:
                        wn -= 1
                    wavespans.append((ci, wn))
                    ci += wn
                ABUFS = 5

                def emit_wave_dma(w):
                    ci, wn = wavespans[w]
                    at = keep.tile([128, WV, 2, 128], f8, tag="a",
                                   bufs=ABUFS, name="awave")
                    flat = at[:, 0:wn, :, :].rearrange("p w i d -> p (w i d)")
                    n = wn * 256
                    s1, s2 = 43, 86
                    nc.sync.dma_start(flat[0:s1, :],
                                      a_d[0:s1, ci * 256:ci * 256 + n])
                    nc.scalar.dma_start(flat[s1:s2, :],
                                        a_d[s1:s2, ci * 256:ci * 256 + n])
                    nc.gpsimd.dma_start(flat[s2:128, :],
                                        a_d[s2:128, ci * 256:ci * 256 + n])
                    return at

                # prefetch first ABUFS waves, THEN the AG-gated h0a loads so
                # the gpsimd queue isn't blocked ahead of them
                atiles = {w: emit_wave_dma(w) for w in range(ABUFS)}
                for r in range(NCORES):
                    nc.gpsimd.dma_start(
                        h0a[:, r * NT:r * NT + H0T, :]
                        .rearrange("p t m -> p (t m)"),
                        ag_out0[r * 128:(r + 1) * 128, :])
                for r in range(NCORES):
                    nc.gpsimd.dma_start(
                        h0a[:, r * NT + H0T:(r + 1) * NT, :]
                        .rearrange("p t m -> p (t m)"),
                        ag_out1[r * 128:(r + 1) * 128, :])

                for w, (ci, wn) in enumerate(wavespans):
                    at = atiles.pop(w)
                    if w + ABUFS < len(wavespans):
                        atiles[w + ABUFS] = emit_wave_dma(w + ABUFS)
                    for j in range(wn):
                        a, t, h, first, last = meta[ci + j]
                        if first:
                            cur_ps[(a, t)] = psm.tile(
                                [128, HID], f32, tag="sc", name="scps")
                        sp = srcpair[ci + j]
                        nc.tensor.matmul(
                            cur_ps[(a, t)][:], at[:, j, :, :],
                            h0a[:, 2 * sp:2 * sp + 2, :],
                            perf_mode=mybir.MatmulPerfMode.DoubleRow,
                            start=first, stop=last)
                        if last:
                            nc.vector.tensor_copy(
                                h12[:, a, t, :], cur_ps[(a, t)][:])
                            for hf in range(2):
                                pst = ptr.tile([128, 128], bf16, tag="tr")
                                nc.tensor.transpose(
                                    pst[:],
                                    h12[:, a, t, 128 * hf:128 * (hf + 1)],
                                    ws_sb[:, OI:OI + 128])
                                nc.vector.tensor_copy(
                                    hT[:, 2 + 4 * h + 2 * a + hf,
                                       128 * t:128 * (t + 1)], pst[:])
                        if ci + j == tile_last[t]:
                            po = pmm.tile([128, OUT_C], f32, tag="mm")
                            nc.tensor.matmul(
                                po[:], ws_sb[0:1, OO:OO + 128],
                                ws_sb[0:1, OB:OB + OUT_C],
                                start=True, stop=False)
                            for i_k in range(10):
                                wk = i_k if i_k < 2 else 2 + (i_k - 2) % 4
                                nc.tensor.matmul(
                                    po[:], hT[:, i_k, 128 * t:128 * (t + 1)],
                                    ws_sb[:, OWO + wk * OUT_C:
                                          OWO + (wk + 1) * OUT_C],
                                    start=False, stop=(i_k == 9))
                            o_sb = keep.tile([128, OUT_C], f32, tag="osb", bufs=2)
                            nc.vector.tensor_copy(o_sb[:], po[:])
                            orows = min(128, ROWS - 128 * t)
                            nc.sync.dma_start(
                                out[128 * t:128 * t + orows, :], o_sb[:orows, :])
    nc.compile()
    return nc


def kernel(x, adj1_rows, adj1_cols, adj1_vals, adj2_rows, adj2_cols, adj2_vals,
           W1, W_out, b_out):
    global LAST_EXEC_NS, LAST_RESULTS
    _install_trace_shim()
    x_bf = np.ascontiguousarray(np.asarray(x, np.float32)).astype(bfnp)
    W1 = np.ascontiguousarray(np.asarray(W1, np.float32))
    W_out = np.ascontiguousarray(np.asarray(W_out, np.float32))
    b_out = np.asarray(b_out, np.float32).ravel()

    w1_blob = np.ascontiguousarray(
        W1.reshape(KT, 128, HID).transpose(1, 0, 2).reshape(128, KT * HID)
    ).astype(bfnp)
    ws = np.zeros((128, BLOBW), np.float32)
    ws[:, OWO:OWO + 6 * OUT_C] = \
        W_out.reshape(6, 128, OUT_C).transpose(1, 0, 2).reshape(128, 6 * OUT_C)
    ws[0, OB:OB + OUT_C] = b_out
    ws[0, OO:OO + 128] = 1.0
    ws[:, OI:OI + 128] = np.eye(128, dtype=np.float32)
    ws = ws.astype(bfnp)

    adj = [(np.asarray(adj1_rows, np.int64), np.asarray(adj1_cols, np.int64),
            np.asarray(adj1_vals, np.float32)),
           (np.asarray(adj2_rows, np.int64), np.asarray(adj2_cols, np.int64),
            np.asarray(adj2_vals, np.float32))]
    in_maps = []
    for c in range(NCORES):
        in_maps.append({
            "xt": _prep_xt(x_bf, c), "w1": w1_blob, "ws": ws,
            "ablob": _prep_adj(adj, c),
        })

    nc = _build()
    try:
        res = bass_utils.run_bass_kernel_spmd(
            nc, in_maps, core_ids=list(range(NCORES)), trace=True,
            trace_cores=[0])
    except Exception:
        res = bass_utils.run_bass_kernel_spmd(
            nc, in_maps, core_ids=list(range(NCORES)), trace=False)
    LAST_EXEC_NS = res.exec_time_ns
    LAST_RESULTS = res
    return np.concatenate([res.results[c]["out"] for c in range(NCORES)], axis=0)


# revision 35
# speedup vs baseline: 3.4325x; 3.4325x over previous
"""H2GCN forward on 8 Trainium2 NeuronCores — dense fp8 DoubleRow SpMM.

out = concat([h0, A1@h0, A2@h0], 1) @ W_out + b_out,  h0 = x @ W1

Data-parallel over destination nodes (1250 rows/core). Per core:
  h0 = x_shard @ W1 in bf16 (t-major, xT k-tiles streamed from DRAM),
  AllGather h0 in fp8e4 (two halves: rows 0-511 / 512-1249),
  SpMM as dense-block matmuls in fp8 DoubleRow perf mode: each instruction
  contracts a PAIR of 128-row source tiles (256 rows) against the dest-tile
  selection block at 0.5 cycles/row — 4x the bf16 dense rate, and the A
  matrices are 26 MB instead of 52 MB.  Half-0/half-1 source partials are
  kept separate (hT k-slices 2-5 / 6-9) so half-0 matmuls run while the
  second AllGather is still in flight; the final out GEMM sums both.
"""
import sys
import types

for _p in ("/opt/trn_rl_repo", "/root/.axon_site", "/root/.axon_site/_ro/trn_rl_repo",
           "/root/.axon_site/_ro/pypackages"):
    if _p not in sys.path:
        sys.path.append(_p)

import numpy as np
import ml_dtypes
import concourse.bass as bass
import concourse.bacc as bacc
import concourse.mybir as mybir
import concourse.tile as tile
from concourse import bass_utils

N, IN_C, HID, OUT_C = 10000, 2048, 256, 256
NCORES = 8
ROWS = N // NCORES          # 1250
NT = 10                     # dest tiles of 128 (last has 98 valid rows)
KT = IN_C // 128            # 16
H0T = 4                     # AG half 0: source tiles 0-3 of each shard
H1T = 6                     # half 1: tiles 4-9 incl. zero pad rows 1250-1279
PAIRS = ([0, 1], [2, 3, 4])  # DoubleRow source-tile pairs per AG half
NPAIR = 5                   # source-tile pairs per shard (DoubleRow)
WV = 64                     # chunks per A-stream wave (16KB/partition descs)

f32 = mybir.dt.float32
bf16 = mybir.dt.bfloat16
f8 = mybir.dt.float8e4
bfnp = ml_dtypes.bfloat16
f8np = ml_dtypes.float8_e4m3

# small weights blob (bf16): Wout k-tiles, bias, ones, identity
OWO, OB, OO, OI = 0, 6 * OUT_C, 6 * OUT_C + OUT_C, 6 * OUT_C + OUT_C + 128
BLOBW = OI + 128

LAST_EXEC_NS = None
LAST_RESULTS = None


def _install_trace_shim():
    try:
        import antenv.axon_hooks  # noqa: F401
        return
    except ImportError:
        pass
    try:
        import antenv
        from trn_agent_boot.trn_boot import _ntff_profile_via_ctypes
        hook = _ntff_profile_via_ctypes("/opt/axon/libaxon_pjrt.so")
        mod = types.ModuleType("antenv.axon_hooks")
        mod.get_axon_ntff_profile_hook = lambda: hook
        mod.set_axon_ntff_profile_hook = lambda h: None
        sys.modules["antenv.axon_hooks"] = mod
        antenv.axon_hooks = mod
    except Exception:
        pass


def _chunk_meta():
    """Global chunk order: half-major, dest-tile, adj, then (core, pair).
    half 0 = pairs 0-1 (src rows 0-511), half 1 = pairs 2-4 (rows 512-1279).
    Returns (meta, tile_last): meta[i] = (a, t, h, first, last)."""
    meta = []
    tile_last = {}
    for h in (0, 1):
        pairs = PAIRS[h]
        for t in range(NT):
            for a in (0, 1):
                n = NCORES * len(pairs)
                for i in range(n):
                    meta.append((a, t, h, i == 0, i == n - 1))
                    tile_last[t] = len(meta) - 1
    return meta, tile_last


def _prep_xt(x_bf, core):
    """xT blob: block (t, k) at cols (t*KT + k)*128, [128 feat, 128 rows]."""
    xt = np.zeros((128, NT * KT * 128), bfnp)
    xsh = x_bf[core * ROWS:(core + 1) * ROWS]
    for t in range(NT):
        rows = min(128, ROWS - t * 128)
        b = xsh[t * 128:t * 128 + rows].T.reshape(KT, 128, rows)
        for k in range(KT):
            xt[:, (t * KT + k) * 128:(t * KT + k) * 128 + rows] = b[k]
    return xt


def _prep_adj(adj, core):
    """Dense A^T per adjacency for this core's dest shard in fp8, laid out in
    global chunk order: chunk = (a,t,h,core r,pair j2) -> [128, 2, 128]."""
    dense = []
    for (rows, cols, vals) in adj:
        lo = core * ROWS
        m = (rows >= lo) & (rows < lo + ROWS)
        r, c, v = rows[m] - lo, cols[m], vals[m]
        # src index in padded tile space: core*1280 + (col % 1250)
        src = (c // ROWS) * (NT * 128) + (c % ROWS)
        A = np.zeros((NCORES * NT * 128, ROWS), np.float32)
        np.add.at(A, (src, r), v)
        dense.append(A.astype(f8np))
    meta, _ = _chunk_meta()
    blob = np.zeros((128, len(meta) * 256), f8np)
    pos = {}
    cnt = {}
    for i, (a, t, h, _, _) in enumerate(meta):
        k = (a, t, h)
        j = cnt.get(k, 0)
        cnt[k] = j + 1
        pos[(a, t, h, j)] = i
    for a in (0, 1):
        A = dense[a]
        for h in (0, 1):
            pairs = PAIRS[h]
            for t in range(NT):
                for rj, (rr, j2) in enumerate(
                        (rr, j2) for rr in range(NCORES) for j2 in pairs):
                    i = pos[(a, t, h, rj)]
                    for half_pair in range(2):
                        s = rr * NT + 2 * j2 + half_pair
                        blk = A[s * 128:(s + 1) * 128,
                                t * 128:min((t + 1) * 128, ROWS)]
                        blob[:, i * 256 + half_pair * 128:
                             i * 256 + half_pair * 128 + blk.shape[1]] = blk
    return blob


def _build():
    meta, tile_last = _chunk_meta()
    nchunks = len(meta)
    nc = bacc.Bacc("TRN2", target_bir_lowering=False, debug=False,
                   num_devices=NCORES)
    xt_d = nc.dram_tensor("xt", [128, NT * KT * 128], bf16, kind="ExternalInput")
    w1_d = nc.dram_tensor("w1", [128, KT * HID], bf16, kind="ExternalInput")
    ws_d = nc.dram_tensor("ws", [128, BLOBW], bf16, kind="ExternalInput")
    a_d = nc.dram_tensor("ablob", [128, nchunks * 256], f8, kind="ExternalInput")
    out = nc.dram_tensor("out", [ROWS, OUT_C], f32, kind="ExternalOutput")

    with tile.TileContext(nc) as tc:
        with tc.tile_pool(name="keep", bufs=1) as keep, \
             tc.tile_pool(name="dram", bufs=1, space="DRAM") as dram, \
             tc.tile_pool(name="pmm", bufs=2, space="PSUM") as pmm, \
             tc.tile_pool(name="psm", bufs=3, space="PSUM") as psm, \
             tc.tile_pool(name="ptr", bufs=2, space="PSUM") as ptr:

            w1_sb = keep.tile([128, KT * HID], bf16)
            ws_sb = keep.tile([128, BLOBW], bf16)
            ag_sb = keep.tile([128, NT, HID], f8)
            h0a = keep.tile([128, NCORES * NT, HID], f8)
            # hT k-slices: 0,1 = h0; 2..5 = h1,h2 half-0 partials;
            # 6..9 = h1,h2 half-1 partials (out GEMM sums both)
            hT = keep.tile([128, 10, NT * 128], bf16)
            h12 = keep.tile([128, 2, NT, HID], bf16)
            nc.sync.dma_start(w1_sb[:], w1_d[:])
            nc.sync.dma_start(ws_sb[:], ws_d[:])

            # partition-major AG payload: row p = SBUF partition p's tiles,
            # so every DMA on both sides is contiguous (1-1.5KB descriptors)
            ag_in0 = dram.tile([128, H0T * HID], f8)
            ag_in1 = dram.tile([128, H1T * HID], f8)
            ag_out0 = dram.tile([NCORES * 128, H0T * HID], f8, addr_space="Shared")
            ag_out1 = dram.tile([NCORES * 128, H1T * HID], f8, addr_space="Shared")
            h0bf = keep.tile([128, NT, HID], bf16)

            # ---- phase A: h0 = x @ W1 (bf16), t-major with streamed xT ----
            with nc.named_scope("h0_gemm"):
                for t in range(NT):
                    xtile = keep.tile([128, KT * 128], bf16, tag="xt", bufs=2)
                    nc.sync.dma_start(
                        xtile[:], xt_d[:, t * KT * 128:(t + 1) * KT * 128])
                    ps = pmm.tile([128, HID], f32, tag="mm")
                    for k in range(KT):
                        nc.tensor.matmul(
                            ps[:], xtile[:, k * 128:(k + 1) * 128],
                            w1_sb[:, k * HID:(k + 1) * HID],
                            start=(k == 0), stop=(k == KT - 1))
                    nc.vector.tensor_copy(h0bf[:, t, :], ps[:])
                    nc.vector.tensor_copy(ag_sb[:, t, :], ps[:])
                    if t == H0T - 1:
                        nc.scalar.dma_start(
                            ag_in0[:],
                            ag_sb[:, 0:H0T, :].rearrange("p t m -> p (t m)"))
                    if t == NT - 1:
                        nc.scalar.dma_start(
                            ag_in1[:],
                            ag_sb[:, H0T:NT, :].rearrange("p t m -> p (t m)"))

            # ---- phase B: AllGather h0 (fp8), two halves ----
            with nc.named_scope("allgather"):
                nc.gpsimd.collective_compute(
                    "AllGather", mybir.AluOpType.bypass,
                    replica_groups=[list(range(NCORES))],
                    ins=[ag_in0.opt()], outs=[ag_out0.opt()])
                nc.gpsimd.collective_compute(
                    "AllGather", mybir.AluOpType.bypass,
                    replica_groups=[list(range(NCORES))],
                    ins=[ag_in1.opt()], outs=[ag_out1.opt()])

            # ---- deferred h0 transposes: run on PE during the AG wait ----
            with nc.named_scope("h0t"):
                for t in range(NT):
                    for hf in range(2):
                        pst = ptr.tile([128, 128], bf16, tag="tr")
                        nc.tensor.transpose(
                            pst[:], h0bf[:, t, 128 * hf:128 * (hf + 1)],
                            ws_sb[:, OI:OI + 128])
                        nc.vector.tensor_copy(
                            hT[:, hf, 128 * t:128 * (t + 1)], pst[:])

            # ---- phase C: dense SpMM, fp8 DoubleRow, streamed A ----
            with nc.named_scope("spmm"):
                cur_ps = {}
                srcpair = {}
                cnt = {}
                for i, (a, t, h, _, _) in enumerate(meta):
                    pairs = PAIRS[h]
                    j = cnt.get((a, t, h), 0)
                    cnt[(a, t, h)] = j + 1
                    rr, j2 = j // len(pairs), pairs[j % len(pairs)]
                    srcpair[i] = rr * NPAIR + j2
                # wave boundaries (don't cross the half boundary)
                wavespans = []
                ci = 0
                while ci < len(meta):
                    wn = min(WV, len(meta) - ci)
                    hcur = meta[ci][2]
                    while meta[ci + wn - 1][2] != hcur:
                        wn -= 1
                    wavespans.append((ci, wn))
                    ci += wn
                for r in range(NCORES):
                    nc.gpsimd.dma_start(
                        h0a[:, r * NT:r * NT + H0T, :]
                        .rearrange("p t m -> p (t m)"),
                        ag_out0[r * 128:(r + 1) * 128, :])
                for r in range(NCORES):
                    nc.gpsimd.dma_start(
                        h0a[:, r * NT + H0T:(r + 1) * NT, :]
                        .rearrange("p t m -> p (t m)"),
                        ag_out1[r * 128:(r + 1) * 128, :])

                for w, (ci, wn) in enumerate(wavespans):
                    at = keep.tile([128, WV, 2, 128], f8, tag="a",
                                   bufs=6, name="awave")
                    eng = nc.scalar if w % 2 == 0 else nc.sync
                    eng.dma_start(
                        at[:, 0:wn, :, :].rearrange("p w i d -> p (w i d)"),
                        a_d[:, ci * 256:(ci + wn) * 256])
                    for j in range(wn):
                        a, t, h, first, last = meta[ci + j]
                        if first:
                            cur_ps[(a, t)] = psm.tile(
                                [128, HID], f32, tag="sc", name="scps")
                        sp = srcpair[ci + j]
                        nc.tensor.matmul(
                            cur_ps[(a, t)][:], at[:, j, :, :],
                            h0a[:, 2 * sp:2 * sp + 2, :],
                            perf_mode=mybir.MatmulPerfMode.DoubleRow,
                            start=first, stop=last)
                        if last:
                            nc.vector.tensor_copy(
                                h12[:, a, t, :], cur_ps[(a, t)][:])
                            for hf in range(2):
                                pst = ptr.tile([128, 128], bf16, tag="tr")
                                nc.tensor.transpose(
                                    pst[:],
                                    h12[:, a, t, 128 * hf:128 * (hf + 1)],
                                    ws_sb[:, OI:OI + 128])
                                nc.vector.tensor_copy(
                                    hT[:, 2 + 4 * h + 2 * a + hf,
                                       128 * t:128 * (t + 1)], pst[:])
                        if ci + j == tile_last[t]:
                            po = pmm.tile([128, OUT_C], f32, tag="mm")
                            nc.tensor.matmul(
                                po[:], ws_sb[0:1, OO:OO + 128],
                                ws_sb[0:1, OB:OB + OUT_C],
                                start=True, stop=False)
                            for i_k in range(10):
                                wk = i_k if i_k < 2 else 2 + (i_k - 2) % 4
                                nc.tensor.matmul(
                                    po[:], hT[:, i_k, 128 * t:128 * (t + 1)],
                                    ws_sb[:, OWO + wk * OUT_C:
                                          OWO + (wk + 1) * OUT_C],
                                    start=False, stop=(i_k == 9))
                            o_sb = keep.tile([128, OUT_C], f32, tag="osb", bufs=2)
                            nc.vector.tensor_copy(o_sb[:], po[:])
                            orows = min(128, ROWS - 128 * t)
                            nc.sync.dma_start(
                                out[128 * t:128 * t + orows, :], o_sb[:orows, :])
    nc.compile()
    return nc


def kernel(x, adj1_rows, adj1_cols, adj1_vals, adj2_rows, adj2_cols, adj2_vals,
           W1, W_out, b_out):
    global LAST_EXEC_NS, LAST_RESULTS
    _install_trace_shim()
    x_bf = np.ascontiguousarray(np.asarray(x, np.float32)).astype(bfnp)
    W1 = np.ascontiguousarray(np.asarray(W1, np.float32))
    W_out = np.ascontiguousarray(np.asarray(W_out, np.float32))
    b_out = np.asarray(b_out, np.float32).ravel()

    w1_blob = np.ascontiguousarray(
        W1.reshape(KT, 128, HID).transpose(1, 0, 2).reshape(128, KT * HID)
    ).astype(bfnp)
    ws = np.zeros((128, BLOBW), np.float32)
    ws[:, OWO:OWO + 6 * OUT_C] = \
        W_out.reshape(6, 128, OUT_C).transpose(1, 0, 2).reshape(128, 6 * OUT_C)
    ws[0, OB:OB + OUT_C] = b_out
    ws[0, OO:OO + 128] = 1.0
    ws[:, OI:OI + 128] = np.eye(128, dtype=np.float32)
    ws = ws.astype(bfnp)

    adj = [(np.asarray(adj1_rows, np.int64), np.asarray(adj1_cols, np.int64),
            np.asarray(adj1_vals, np.float32)),
           (np.asarray(adj2_rows, np.int64), np.asarray(adj2_cols, np.int64),
            np.asarray(adj2_vals, np.float32))]
    in_maps = []
    for c in range(NCORES):
        in_maps.append({
            "xt": _prep_xt(x_bf, c), "w1": w1_blob, "ws": ws,
            "ablob": _prep_adj(adj, c),
        })

    nc = _build()
    try:
        res = bass_utils.run_bass_kernel_spmd(
            nc, in_maps, core_ids=list(range(NCORES)), trace=True,
            trace_cores=[0])
    except Exception:
        res = bass_utils.run_bass_kernel_spmd(
            nc, in_maps, core_ids=list(range(NCORES)), trace=False)
    LAST_EXEC_NS = res.exec_time_ns
    LAST_RESULTS = res
    return np.concatenate([res.results[c]["out"] for c in range(NCORES)], axis=0)
